# revision 35
# baseline (speedup 1.0000x reference)
"""Biaffine attention kernel for Trainium2, data-parallel over 8 NeuronCores.

Math (per batch b):
    xp = Wf @ x[b] + bf          (128, L)
    yp = Wa @ y[b] + ba          (128, L)
    scores = xp @ yp.T           (128, 128)   contraction over L
    attn = softmax(scores, -1) / sqrt(L)
    out[b] = attn @ (xp + yp)    (128, L)

Distribution: batch dim (32) sharded 4-per-core across 8 cores; weights
replicated. No collectives.

Dataflow (fp16 I/O, "route E2b"):
  - x/y are cast to fp16 on the host (HBM reads 16 MiB/core); the output is
    written fp16 (8 MiB/core) and cast back on the host. DMA floor ~70us.
  - Projections are computed TRANSPOSED per 128-col chunk:
    xpT_c[l,o] = matmul(lhsT=x_c[f,l], rhs=WfT[f,o]). The PSUM->SBUF CAST
    evacuations (DVE for xpT, ScalarE for ypT, 512 cols each) directly
    produce the scores matmul operands - no separate transpose pass.
  - zT = xpT + ypT on the Pool engine (SBUF-only; Pool cannot read PSUM);
    z natural is rebuilt with one fp16 PE transpose per chunk (8 chunks per
    fp16 PSUM bank - fp16 PSUM reads are ~2x cheaper than fp32 CASTs); the
    z evacuation fuses (z + bf + ba) * (1/sqrt(L)) in a single DVE
    tensor_scalar, which also removes the softmax 1/sqrt(L) rescale.
  - Biases enter scores exactly via a HOST-computed rank-2 correction
    appended as a final k=2 matmul into the scores PSUM accumulation.
  - out = attnT.T @ z, one 512-col matmul per chunk.
  - Batches are software-pipelined: out(b-1) matmuls/evacs interleave into
    phase1(b) one per group, the attnT transpose is deferred past the
    softmax chain, and the scores PSUM is double-buffered, so the PE never
    waits on softmax or on laggard evacuation queues.
  - DMA-issue discipline: each dma_start costs ~600ns of HWDGE sequencer
    time, so loads/stores are split across the SP and ACT rings, stores are
    merged to 2048-col tiles, and the hot first-tile halves + weights issue
    before the cold constants.
"""

import numpy as np

P = 128
L = 8192
B = 32
NCORES = 8
BPC = B // NCORES  # batches per core
SQRT_L = float(np.sqrt(float(L)))

IN_TILE = 4096  # HBM->SBUF dma tile (1 MiB fp16)
OUT_TILE = 512  # out matmul free dim / store tile
GRP = 4  # proj chunks per PSUM bank (512 cols)
SCORES_LAG = 2  # groups between proj-evac and scores mms
ZBACK_LAG = 5  # groups between proj-evac and z transpose-backs


def _patch_tail_drain(tile, mybir, ScopedClock):
    """This container's walrus rejects >1 sync wait on the kernel-tail Drain
    (setupSyncWait: 'Too many sync wait commands'). Spread the tail-drain
    waits across a chain of drains, one wait each."""
    if getattr(tile.TileContext, "_drain_split_patched", False):
        return

    def _split_drain_and_barrier(self, tick_clock, wait_clock):
        nc = self.nc
        drain_inst = nc.sync.drain()
        wait_clock.add_sem_waits(
            drain_inst.ins, ScopedClock({None: tick_clock.global_clock})
        )
        si = drain_inst.ins.sync_info
        if si is not None and si.on_wait is not None and len(si.on_wait) > 1:
            waits = list(si.on_wait)
            si.on_wait = waits[:1]
            for w in waits[1:]:
                extra = nc.sync.drain()
                esi = extra.ins.sync_info
                if esi is None:
                    extra.ins.sync_info = mybir.SyncInfo(on_wait=[w], on_update=[])
                else:
                    ow = list(esi.on_wait) if esi.on_wait else []
                    ow.append(w)
                    esi.on_wait = ow
        nc.all_engine_barrier()
        assert self.sems is not None
        popped = nc._tile_sem_poison_stack.pop()
        assert popped is self._sem_poison
        nc.clear_and_free_semaphores(list(self.sems.allocated().values()))
        nc.all_engine_barrier()

    tile.TileContext._drain_and_barrier = _split_drain_and_barrier
    tile.TileContext._drain_split_patched = True


def _split_excess_waits(nc, mybir, max_waits=1):
    """Walrus in this container rejects instructions carrying more than a
    couple of sync waits ('Too many sync wait commands'). Hoist excess waits
    onto dedicated same-engine NoOps inserted just before the instruction."""
    ctr = 0
    for blk in nc.m.functions[0].blocks:
        new_insts = []
        for inst in blk.instructions:
            si = inst.sync_info
            if si is not None and si.on_wait and len(si.on_wait) > max_waits:
                waits = list(si.on_wait)
                excess, keep = waits[:-max_waits], waits[-max_waits:]
                si.on_wait = keep
                for i in range(0, len(excess), max_waits):
                    ctr += 1
                    nop = mybir.InstNoOp(
                        name=f"I-waitsplit-{ctr}",
                        sync_info=mybir.SyncInfo(
                            on_wait=excess[i : i + max_waits], on_update=[]
                        ),
                        bass_nofuse=True,
                        engine=inst.engine,
                    )
                    nc.register_instruction(nop)
                    new_insts.append(nop)
            new_insts.append(inst)
        blk.instructions = new_insts


def build_nc(bpc=BPC, seq=L):
    import concourse.bass as bass
    import concourse.mybir as mybir
    import concourse.tile as tile
    from concourse.masks import make_identity
    from concourse.vector_clock import ScopedClock

    _patch_tail_drain(tile, mybir, ScopedClock)

    f32 = mybir.dt.float32
    f16 = mybir.dt.float16
    AF = mybir.ActivationFunctionType
    ALU = mybir.AluOpType
    AX = mybir.AxisListType

    sqrt_l = float(np.sqrt(float(seq)))
    nchunk = seq // P  # 128-col proj chunks per batch (64)
    ngrp = nchunk // GRP  # 4-chunk groups (16)
    grp_per_tile = IN_TILE // (GRP * P)  # groups per input dma tile (8)
    nout = seq // OUT_TILE
    GW = GRP * P  # group width in cols (512)

    nc = bass.Bass("TRN2", target_bir_lowering=False, debug=False)
    x_d = nc.dram_tensor("x", [bpc, P, seq], f16, kind="ExternalInput").ap()
    y_d = nc.dram_tensor("y", [bpc, P, seq], f16, kind="ExternalInput").ap()
    wft_d = nc.dram_tensor("wft", [P, P], f16, kind="ExternalInput").ap()
    wat_d = nc.dram_tensor("wat", [P, P], f16, kind="ExternalInput").ap()
    bzv_d = nc.dram_tensor("bzv", [P], f32, kind="ExternalInput").ap()
    l2_d = nc.dram_tensor("l2", [2, bpc * P], f16, kind="ExternalInput").ap()
    r2_d = nc.dram_tensor("r2", [2, bpc * P], f16, kind="ExternalInput").ap()
    out_d = nc.dram_tensor("out", [bpc, P, seq], f16, kind="ExternalOutput").ap()

    with tile.TileContext(nc) as tc:
        with (
            tc.tile_pool(name="consts", bufs=1) as consts,
            tc.tile_pool(name="xin", bufs=6) as xin_pool,
            tc.tile_pool(name="acts", bufs=1) as acts_pool,
            tc.tile_pool(name="zp", bufs=2) as z_pool,
            tc.tile_pool(name="sm", bufs=2) as sm_pool,
            tc.tile_pool(name="outs", bufs=4) as out_pool,
            tc.tile_pool(name="pp", bufs=3, space="PSUM") as psum_p,
            tc.tile_pool(name="pz", bufs=2, space="PSUM") as psum_z,
            tc.tile_pool(name="psc", bufs=2, space="PSUM") as psum_sc,
            tc.tile_pool(name="pout", bufs=1, space="PSUM") as psum_out,
        ):
            # ---- constants FIRST: the weight loads are tiny and gate the
            # very first projection matmul ----
            # hot-path first: wf + first x half on SP ring, wa + first y half
            # on ACT ring; cold consts (bzv/l2/r2) queue behind them
            preloaded = {}
            x_t0 = xin_pool.tile([P, IN_TILE], f16, tag="x_t", name="x_t")
            y_t0 = xin_pool.tile([P, IN_TILE], f16, tag="y_t", name="y_t")
            hq = 2048
            wf_sb = consts.tile([P, P], f16)
            nc.sync.dma_start(wf_sb, wft_d)
            wa_sb = consts.tile([P, P], f16)
            nc.scalar.dma_start(wa_sb, wat_d)
            nc.sync.dma_start(x_t0[:, :hq], x_d[0, :, :hq])
            nc.scalar.dma_start(y_t0[:, :hq], y_d[0, :, :hq])
            nc.sync.dma_start(x_t0[:, hq:], x_d[0, :, hq:IN_TILE])
            nc.scalar.dma_start(y_t0[:, hq:], y_d[0, :, hq:IN_TILE])

            preloaded[0] = (x_t0, y_t0)
            x_t1 = xin_pool.tile([P, IN_TILE], f16, tag="x_t", name="x_t")
            y_t1 = xin_pool.tile([P, IN_TILE], f16, tag="y_t", name="y_t")
            hs = slice(IN_TILE, 2 * IN_TILE)
            nc.sync.dma_start(x_t1, x_d[0, :, hs])
            nc.scalar.dma_start(y_t1, y_d[0, :, hs])
            preloaded[1] = (x_t1, y_t1)
            bzv_sb = consts.tile([P, 1], f32)
            nc.scalar.dma_start(bzv_sb, bzv_d.rearrange("(p o) -> p o", o=1))
            l2_sb = consts.tile([2, bpc * P], f16)
            nc.scalar.dma_start(l2_sb, l2_d)
            r2_sb = consts.tile([2, bpc * P], f16)
            nc.scalar.dma_start(r2_sb, r2_d)

            ids = consts.tile([P, P], f16)
            make_identity(nc, ids)

            ctx = {}  # per-batch tiles needed by the lagged out phase

            def emit_phase1(b):
                xpT = acts_pool.tile([P, seq], f16, tag="xpT", name="xpT")
                ypT = acts_pool.tile([P, seq], f16, tag="ypT", name="ypT")
                zT = acts_pool.tile([P, seq], f16, tag="zT", name="zT")
                z = z_pool.tile([P, seq], f16, tag="z", name="z")
                ps_t = psum_sc.tile([P, P], f32, tag="ps", name="ps_t")
                x_t = y_t = None

                def emit_scores(g):
                    for j in range(GRP):
                        c = GRP * g + j
                        cs = slice(c * P, (c + 1) * P)
                        nc.tensor.matmul(
                            ps_t, xpT[:, cs], ypT[:, cs],
                            start=(c == 0), stop=False,
                        )

                def emit_zadd(g):
                    # zT = xpT + ypT, SBUF fp16, on Pool: its 1151ns/add
                    # total throughput fits the batch and the deep ZBACK_LAG
                    # buffers the ~1.3us group cadence. Batch 0 runs faster
                    # (~1.0us/group, no out-interleave) than Pool can add, so
                    # there DVE (423ns/add, idle without out-evacs) takes
                    # every other add.
                    gs = slice(g * GW, (g + 1) * GW)
                    eng = nc.vector if b == 0 and g % 2 == 1 else nc.gpsimd
                    eng.tensor_add(zT[:, gs], xpT[:, gs], ypT[:, gs])

                pz_banks = {}

                def emit_zback(g):
                    # one fp16 transpose per chunk: z_c = T(zT_c)
                    bk = g // 2
                    if bk not in pz_banks:
                        pz_banks[bk] = psum_z.tile(
                            [P, 2 * GW], f16, tag="pz", name="pz_t"
                        )
                    pzt = pz_banks[bk]
                    for j in range(GRP):
                        c = GRP * g + j
                        cs = slice(c * P, (c + 1) * P)
                        zi = c % (2 * GRP)
                        zs = slice(zi * P, (zi + 1) * P)
                        nc.tensor.matmul(
                            pzt[:, zs], zT[:, cs], ids,
                            is_transpose=True, start=True, stop=True,
                        )

                def emit_zevac(bk):
                    # evac the completed 8-chunk fp16 bank (groups 2bk, 2bk+1)
                    # with the fused (bf+ba) bias; DVE is ~1.6x faster here
                    gs = slice(2 * bk * GW, (2 * bk + 2) * GW)
                    pzt = pz_banks.pop(bk)
                    nc.vector.tensor_scalar(
                        out=z[:, gs], in0=pzt, scalar1=bzv_sb,
                        scalar2=1.0 / sqrt_l, op0=ALU.add, op1=ALU.mult,
                    )

                for g in range(ngrp):
                    if g % grp_per_tile == 0:
                        h = g // grp_per_tile
                        if b == 0 and h in preloaded:
                            x_t, y_t = preloaded[h]
                        else:
                            x_t = xin_pool.tile([P, IN_TILE], f16, tag="x_t", name="x_t")
                            y_t = xin_pool.tile([P, IN_TILE], f16, tag="y_t", name="y_t")
                            hs = slice(h * IN_TILE, (h + 1) * IN_TILE)
                            nc.sync.dma_start(x_t, x_d[b, :, hs])
                            nc.scalar.dma_start(y_t, y_d[b, :, hs])
                    if g == 2 and b > 0:
                        emit_attnT(b - 1)
                    # interleave A/B so consecutive matmuls hit different PSUM
                    # banks (same-bank back-to-back writes serialize the PE)
                    ppA = psum_p.tile([P, GW], f32, tag="pp", name="ppA")
                    ppB = psum_p.tile([P, GW], f32, tag="pp", name="ppB")
                    for j in range(GRP):
                        lc = (GRP * g + j) * P - (g // grp_per_tile) * IN_TILE
                        nc.tensor.matmul(
                            ppA[:, j * P : (j + 1) * P],
                            x_t[:, lc : lc + P], wf_sb,
                            start=True, stop=True,
                        )
                        nc.tensor.matmul(
                            ppB[:, j * P : (j + 1) * P],
                            y_t[:, lc : lc + P], wa_sb,
                            start=True, stop=True,
                        )
                    gs = slice(GRP * g * P, GRP * (g + 1) * P)
                    nc.vector.tensor_copy(out=xpT[:, gs], in_=ppA)
                    nc.scalar.activation(
                        out=ypT[:, gs], in_=ppB, func=AF.Identity, bias=0.0
                    )
                    if g >= 1:
                        emit_zadd(g - 1)
                    if g >= SCORES_LAG:
                        emit_scores(g - SCORES_LAG)
                    if g >= ZBACK_LAG:
                        gz = g - ZBACK_LAG
                        if gz % 2 == 1:
                            emit_zback(gz - 1)
                            emit_zback(gz)
                            emit_zevac(gz // 2)
                    if b > 0 and g >= 4:
                        emit_out_chunk(b - 1, g - 4)
                emit_zadd(ngrp - 1)
                for g in range(ngrp - SCORES_LAG, ngrp):
                    emit_scores(g)
                gtail = ngrp - ZBACK_LAG
                if gtail % 2 == 1:
                    gtail -= 1
                for g in range(gtail, ngrp, 2):
                    emit_zback(g)
                    emit_zback(g + 1)
                    emit_zevac(g // 2)
                if b > 0:
                    for oc in range(nout - 4, nout):
                        emit_out_chunk(b - 1, oc)
                # rank-2 bias correction, final accumulation into scores
                bs = slice(b * P, (b + 1) * P)
                nc.tensor.matmul(
                    ps_t, l2_sb[:, bs], r2_sb[:, bs], start=False, stop=True
                )
                ctx[b] = {"z": z, "ps": ps_t}

            def emit_softmax_pre(b):
                # softmax up to attn (SBUF); the attnT transpose is deferred
                # into the next batch's phase1 so the PE never waits on it
                ps_t = ctx[b]["ps"]
                negmx = sm_pool.tile([P, 1], f32, tag="negmx", name="negmx")
                nc.vector.tensor_reduce(
                    out=negmx, in_=ps_t, axis=AX.X, op=ALU.max, negate=True
                )
                e = sm_pool.tile([P, P], f32, tag="e", name="e")
                se = sm_pool.tile([P, 1], f32, tag="se", name="se")
                nc.scalar.activation(
                    out=e, in_=ps_t, func=AF.Exp, bias=negmx, scale=1.0, accum_out=se
                )
                rcp = sm_pool.tile([P, 1], f32, tag="rcp", name="rcp")
                nc.vector.reciprocal(rcp, se)
                attn = sm_pool.tile([P, P], f16, tag="attn", name="attn")
                nc.scalar.activation(
                    out=attn, in_=e, func=AF.Identity, bias=0.0, scale=rcp
                )
                ctx[b]["attn"] = attn

            def emit_attnT(b):
                # pat borrows a bank from the pz pool ring
                pat = psum_z.tile([P, P], f16, tag="pz", name="pat")
                nc.tensor.transpose(pat, ctx[b]["attn"], ids)
                attnT = sm_pool.tile([P, P], f16, tag="attnT", name="attnT")
                nc.vector.tensor_copy(out=attnT, in_=pat)
                ctx[b]["attnT"] = attnT

            def emit_out_chunk(b, oc, borrow_pp=False):
                os_ = slice(oc * OUT_TILE, (oc + 1) * OUT_TILE)
                if borrow_pp:
                    pool, tag = [
                        (psum_out, "po"), (psum_p, "pp"),
                        (psum_p, "pp"), (psum_p, "pp"),
                    ][oc % 4]
                    po_t = pool.tile([P, OUT_TILE], f32, tag=tag, name="po_t")
                else:
                    po_t = psum_out.tile([P, OUT_TILE], f32, tag="po", name="po_t")
                nc.tensor.matmul(
                    po_t, ctx[b]["attnT"], ctx[b]["z"][:, os_], start=True, stop=True
                )
                if oc % 4 == 0:
                    ctx[b]["ot"] = out_pool.tile(
                        [P, 4 * OUT_TILE], f16, tag="ot", name="ot"
                    )
                ot = ctx[b]["ot"]
                half = slice((oc % 4) * OUT_TILE, (oc % 4 + 1) * OUT_TILE)
                if oc % 16 < 7:
                    nc.vector.tensor_copy(out=ot[:, half], in_=po_t)
                else:
                    nc.scalar.activation(
                        out=ot[:, half], in_=po_t, func=AF.Identity, bias=0.0
                    )
                if oc % 4 == 3:
                    ss = slice((oc - 3) * OUT_TILE, (oc + 1) * OUT_TILE)
                    ring = nc.sync if (oc // 4) % 2 == 0 else nc.scalar
                    ring.dma_start(out_d[b, :, ss], ot)

            for b in range(bpc):
                emit_phase1(b)
                emit_softmax_pre(b)
            emit_attnT(bpc - 1)
            for oc in range(nout):
                emit_out_chunk(bpc - 1, oc, borrow_pp=True)

    _split_excess_waits(nc, mybir, max_waits=1)
    return nc


_nc_cache = {}


def _get_nc():
    key = (BPC, L)
    if key not in _nc_cache:
        _nc_cache[key] = build_nc(BPC, L)
    return _nc_cache[key]


def _prep_host(x, y, Wf, bf, Wa, ba):
    """Cast inputs to fp16 and compute the rank-2 scores bias correction."""
    x16 = np.ascontiguousarray(x.astype(np.float16))
    y16 = np.ascontiguousarray(y.astype(np.float16))
    wf16 = Wf.astype(np.float16)
    wa16 = Wa.astype(np.float16)
    # rowsums of the fp16-projected activations (without bias):
    # u0 = Wf @ sum_l x, v0 = Wa @ sum_l y  (fp32 accumulation)
    sx = x16.astype(np.float32).sum(axis=-1)  # (B, 128)
    sy = y16.astype(np.float32).sum(axis=-1)
    u0 = sx @ wf16.astype(np.float32).T  # (B, 128)
    v0 = sy @ wa16.astype(np.float32).T
    nb = x.shape[0]
    l2 = np.empty((nb, 2, P), np.float16)
    r2 = np.empty((nb, 2, P), np.float16)
    l2[:, 0, :] = bf[None, :]
    l2[:, 1, :] = u0
    r2[:, 0, :] = v0 + float(L) * ba[None, :]
    r2[:, 1, :] = ba[None, :]
    wft = np.ascontiguousarray(wf16.T)
    wat = np.ascontiguousarray(wa16.T)
    bzv = (bf + ba).astype(np.float32)
    return x16, y16, wft, wat, bzv, l2, r2


def make_in_maps(x, y, Wf, bf, Wa, ba):
    x = np.asarray(x, dtype=np.float32)
    y = np.asarray(y, dtype=np.float32)
    Wf = np.asarray(Wf, dtype=np.float32)
    bf = np.asarray(bf, dtype=np.float32)
    Wa = np.asarray(Wa, dtype=np.float32)
    ba = np.asarray(ba, dtype=np.float32)
    x16, y16, wft, wat, bzv, l2, r2 = _prep_host(x, y, Wf, bf, Wa, ba)
    in_maps = []
    for c in range(NCORES):
        sl = slice(c * BPC, (c + 1) * BPC)
        in_maps.append(
            {
                "x": np.ascontiguousarray(x16[sl]),
                "y": np.ascontiguousarray(y16[sl]),
                "wft": wft,
                "wat": wat,
                "bzv": bzv,
                "l2": np.ascontiguousarray(
                    l2[sl].transpose(1, 0, 2).reshape(2, BPC * P)
                ),
                "r2": np.ascontiguousarray(
                    r2[sl].transpose(1, 0, 2).reshape(2, BPC * P)
                ),
            }
        )
    return in_maps


def kernel(x, y, Wf, bf, Wa, ba):
    from concourse.bass_utils import run_bass_kernel_spmd

    in_maps = make_in_maps(x, y, Wf, bf, Wa, ba)
    nc = _get_nc()
    res = run_bass_kernel_spmd(nc, in_maps, core_ids=list(range(NCORES)))
    out = np.concatenate([r["out"] for r in res.results], axis=0)
    return np.ascontiguousarray(out.astype(np.float32))


if __name__ == "__main__":
    rng = np.random.default_rng(0)
    inputs = {
        "x": rng.standard_normal((B, P, L), dtype=np.float32),
        "y": rng.standard_normal((B, P, L), dtype=np.float32),
        "Wf": (rng.standard_normal((P, P)) / np.sqrt(P)).astype(np.float32),
        "bf": (rng.standard_normal(P) * 0.02).astype(np.float32),
        "Wa": (rng.standard_normal((P, P)) / np.sqrt(P)).astype(np.float32),
        "ba": (rng.standard_normal(P) * 0.02).astype(np.float32),
    }
    o = kernel(**inputs)
    print(o.shape, o.dtype)
